# revision 7
# baseline (speedup 1.0000x reference)
"""DoSRUconv Trainium2 kernel v4: host-expanded replica images -> pixel-major
conv -> fused act eviction -> bidirectional SRU scan.  bf16 front-end.

Sharding: H split across 8 cores (16 rows each); halo resolved host-side;
no collectives.

v4 change vs v3: the 8 tap-replica DMAs + 3 leftover DMAs per chunk (520 B
strided runs, ~93k descriptors/iter, descriptor-generation-bound at ~4x
below HBM bw) are replaced by host-side expansion of the exact SBUF tile
images.  Each chunk now loads with two fully-contiguous DMAs (mrep: 128
partitions x 17.4 KB, lrep: 49 x 16.4 KB), which spread across all 16 SDMA
engines at line rate.  Same HBM bytes per iteration, ~45x fewer
descriptors.

Per-core pipeline, per (b, 2-row chunk):
  - mrep [128=(tap,c), 33 t-slots, 264] bf16 replica tile, one contiguous
    DMA; lrep [49=(dt,c)+ones, 31, 264] leftover tile (ones row carries the
    bias), one contiguous DMA.
  - conv as PIXEL-major matmuls: lhsT = x view [K, 128w] (stationary),
    rhs = weights [K, 96 gates] (moving, N=96); 3 K=128 dt-blocks + one K=49
    leftover accumulate into PSUM [128w, 96].
  - eviction fused with activations: PSUM -> gp[128, 2r, 96ch, 32t] bf16 with
    tanh (cols 0:32 = Wx,X) / sigmoid (32:96); backward planes ft2/rt2 are
    written time-reversed so both scans run forward.
  - SRU recurrence via tensor_tensor_scan (fp32 internal state) on flattened
    [128, 16x32] views, zero-separator slot 31 between channel segments.
  - output [B, H, W, C, T] f32; host transposes back to [B, C, T, H, W].
"""

import os

import numpy as np
import ml_dtypes

import concourse.bass as bass
import concourse.mybir as mybir
import concourse.tile as tile
from concourse import bacc
from concourse.bass_utils import run_bass_kernel_spmd

F32 = mybir.dt.float32
BF16 = mybir.dt.bfloat16
ALU = mybir.AluOpType
ACTF = mybir.ActivationFunctionType

B, CIN, COUT, T, H, W = 2, 16, 16, 31, 128, 128
NCORES = 8
HSLAB = H // NCORES                  # 16
HC = 2                               # h-rows per chunk
NCHUNK = HSLAB // HC                 # 8
TP, WP = T + 2, W + 2                # 33 t-slices, 130 w
SLOTS = 32                           # t-slots per channel segment
RPITCH = 264                         # replica slot pitch: 2*130 + 4 slack

# 8 main taps (dh, dw), dw-major; leftover tap is (dh=1, dw=1)
MAIN_TAPS = [(dh, dw) for dw in (-1, 0, 1) for dh in (-1, 0, 1)][:8]
# gate plane order [Wx, X, ft, rt, ft2, rt2]; reference order is
# [Wx, ft, ft2, rt, rt2, X]
GPERM = [0, 5, 1, 3, 2, 4]
# column ranges in the 96-gate dim
C_WX, C_X, C_FT, C_RT, C_FT2, C_RT2 = (slice(0, 16), slice(16, 32),
                                       slice(32, 48), slice(48, 64),
                                       slice(64, 80), slice(80, 96))

NCH = B * NCHUNK                     # 16 chunk iterations per core
MRE = TP * RPITCH                    # 8712 elems per mrep partition
LRE = T * RPITCH                     # 8184 elems per lrep partition

# packed single-input layout (element offsets, all bf16)
MR0 = 0
LR0 = MR0 + NCH * 128 * MRE
WM0 = LR0 + NCH * 49 * LRE
WL0 = WM0 + 128 * 3 * 96
NELEM = WL0 + 49 * 96


def _flat2(ap):
    return ap.rearrange("p a b -> p (a b)")


def _rev_last(ap, count, start):
    """View with the last (unit-stride) dim replaced by a reversed run of
    `count` starting at index `start` (descending)."""
    new_ap = [list(d) for d in ap.ap]
    assert new_ap[-1][0] == 1
    new_ap[-1] = [-1, count]
    return bass.AP(tensor=ap.tensor, offset=ap.offset + start, ap=new_ap)


def _lhs_view(rep, slot, r):
    """Matmul lhsT [P, 128w] from a replica tile [P, slots, RPITCH]:
    element w at slot*RPITCH + 2 + 130*r + w."""
    base = rep[:, slot]
    ap = [list(base.ap[0]), [1, W]]
    return bass.AP(tensor=base.tensor, offset=base.offset + 2 + WP * r, ap=ap)


def build_nc():
    nc = bacc.Bacc("TRN2", target_bir_lowering=False, debug=False)

    inp = nc.dram_tensor("inp", [NELEM], BF16, kind="ExternalInput").ap()
    wmain_d = bass.AP(tensor=inp.tensor, offset=WM0,
                      ap=[[288, 128], [96, 3], [1, 96]])
    wleft_d = bass.AP(tensor=inp.tensor, offset=WL0,
                      ap=[[96, 49], [1, 96]])
    # stored [b, h, w, c, t]; host transposes back to [b, c, t, h, w]
    out_d = nc.dram_tensor("out", [B, HSLAB, W, COUT, T], F32,
                           kind="ExternalOutput").ap()

    def src_mrep(pidx):
        return bass.AP(tensor=inp.tensor, offset=MR0 + pidx * 128 * MRE,
                       ap=[[MRE, 128], [1, MRE]])

    def src_lrep(pidx):
        return bass.AP(tensor=inp.tensor, offset=LR0 + pidx * 49 * LRE,
                       ap=[[LRE, 49], [1, LRE]])

    import contextlib
    rep = int(os.environ.get("V3_REPEAT", "1"))
    skip = set(os.environ.get("V3_SKIP", "").split(","))  # ablation probes

    with tile.TileContext(nc) as tc:
        with (
            tc.tile_pool(name="const", bufs=1) as constp,
            tc.tile_pool(name="mrep", bufs=int(os.environ.get("V3_MREP_BUFS", "2"))) as mpool,
            tc.tile_pool(name="lrep", bufs=int(os.environ.get("V3_LREP_BUFS", "2"))) as lpool,
            tc.tile_pool(name="gp", bufs=int(os.environ.get("V3_GP_BUFS", "2"))) as gppool,
            tc.tile_pool(name="scr", bufs=int(os.environ.get("V3_SCR_BUFS", "2"))) as scrp,
            tc.tile_pool(name="mmps", bufs=int(os.environ.get("V3_MM_BUFS", "2")), space="PSUM") as mmpool,
        ):
            wmain_sb = constp.tile([128, 3, 96], BF16)
            nc.sync.dma_start(out=wmain_sb, in_=wmain_d)
            wleft_sb = constp.tile([49, 96], BF16)
            nc.sync.dma_start(out=wleft_sb, in_=wleft_d)

            # timing builds only: repeat the whole body in a HW loop so the
            # per-call axon dispatch overhead amortizes across `rep` runs
            rep_ctx = tc.For_i(0, rep) if rep > 1 else contextlib.nullcontext()
            with rep_ctx:
              for b in range(B):
                for chunk in range(NCHUNK):
                    h0 = chunk * HC
                    pidx = b * NCHUNK + chunk
                    mrep = mpool.tile([128, TP, RPITCH], BF16, tag="mrep")
                    if "dmain" not in skip:
                        nc.sync.dma_start(out=_flat2(mrep),
                                          in_=src_mrep(pidx))
                    lrep = lpool.tile([49, T, RPITCH], BF16, tag="lrep")
                    if "dmain" not in skip:
                        nc.sync.dma_start(out=_flat2(lrep),
                                          in_=src_lrep(pidx))

                    gp = gppool.tile([128, HC, 96, SLOTS], BF16, tag="gp")

                    for cg in range(0, T, 8):
                        gs = min(8, T - cg)
                        ps = mmpool.tile([128, HC, 8, 128], F32, tag="mm")
                        for tt in range(cg, cg + gs):
                            for r in range(HC):
                                pslot = ps[:, r, tt - cg, 0:96]
                                if "mm3" not in skip:
                                    for dt in range(3):
                                        nc.tensor.matmul(
                                            pslot,
                                            _lhs_view(mrep, tt + dt, r),
                                            wmain_sb[:, dt, :],
                                            start=(dt == 0), stop=False)
                                nc.tensor.matmul(
                                    pslot,
                                    _lhs_view(lrep, tt, r),
                                    wleft_sb,
                                    start=("mm3" in skip), stop=True)
                        # fused activation eviction, pixel-major -> gp
                        if "evict" in skip:
                            continue
                        nc.scalar.activation(
                            gp[:, :, 0:32, cg:cg + gs],
                            ps[:, :, 0:gs, 0:32].rearrange(
                                "p r t c -> p r c t"),
                            ACTF.Tanh)
                        nc.scalar.activation(
                            gp[:, :, 32:64, cg:cg + gs],
                            ps[:, :, 0:gs, 32:64].rearrange(
                                "p r t c -> p r c t"),
                            ACTF.Sigmoid)
                        nc.scalar.activation(
                            _rev_last(gp[:, :, 64:96, :], count=gs,
                                      start=30 - cg),
                            ps[:, :, 0:gs, 64:96].rearrange(
                                "p r t c -> p r c t"),
                            ACTF.Sigmoid)

                    bbf = scrp.tile([128, HC, 16, SLOTS], F32, tag="bbf")
                    bbb = scrp.tile([128, HC, 16, SLOTS], F32, tag="bbb")
                    cf = scrp.tile([128, HC, 16, SLOTS], F32, tag="cf")
                    cb = scrp.tile([128, HC, 16, SLOTS], F32, tag="cb")
                    s1 = scrp.tile([128, HC, 16, 31], F32, tag="s1")
                    s2 = scrp.tile([128, HC, 16, 31], F32, tag="s2")
                    dd = scrp.tile([128, HC, 16, 31], BF16, tag="dd")
                    ee = scrp.tile([128, HC, 16, 31], F32, tag="ee")
                    ot = scrp.tile([128, HC, 16, 31], F32, tag="ot")
                    # zero separators (slot 31 of f planes for the scans);
                    # squeeze to <=3 dims (4-dim DVE/Pool ops mis-execute)
                    nc.gpsimd.memset(gp[:, :, 32:96, 31:32].squeeze(), 0.0)
                    nc.gpsimd.memset(bbf[:, :, :, 31:32].squeeze(), 0.0)
                    nc.gpsimd.memset(bbb[:, :, :, 31:32].squeeze(), 0.0)
                    for r in (range(HC) if "scan" not in skip else []):
                        # TensorScalarPtr ops are limited to p + 2 free dims
                        ftv = gp[:, r, C_FT, :]
                        f2v = gp[:, r, C_FT2, :]
                        wxv = gp[:, r, C_WX, :]
                        nc.vector.tensor_scalar_sub(
                            bbf[:, r, :, 0:1], ftv[:, :, 0:1], 1.0)
                        nc.vector.scalar_tensor_tensor(
                            out=bbf[:, r, :, 1:31], in0=ftv[:, :, 1:31],
                            scalar=1.0, in1=wxv[:, :, 1:31],
                            op0=ALU.subtract, op1=ALU.mult)
                        nc.vector.tensor_scalar_sub(
                            bbb[:, r, :, 0:1], f2v[:, :, 0:1], 1.0)
                        nc.vector.scalar_tensor_tensor(
                            out=bbb[:, r, :, 1:31], in0=f2v[:, :, 1:31],
                            scalar=1.0, in1=_rev_last(wxv, 30, 29),
                            op0=ALU.subtract, op1=ALU.mult)
                        nc.vector.tensor_tensor_scan(
                            out=_flat2(cf[:, r]),
                            data0=_flat2(gp[:, r, C_FT, :]),
                            data1=_flat2(bbf[:, r]), initial=0.0,
                            op0=ALU.mult, op1=ALU.subtract)
                        nc.vector.tensor_tensor_scan(
                            out=_flat2(cb[:, r]),
                            data0=_flat2(gp[:, r, C_FT2, :]),
                            data1=_flat2(bbb[:, r]), initial=0.0,
                            op0=ALU.mult, op1=ALU.subtract)
                        # dd[t] = rt[t] + rt2[t]; rt2 is stored reversed
                        nc.vector.tensor_tensor(
                            out=dd[:, r], in0=gp[:, r, C_RT, 0:31],
                            in1=_rev_last(gp[:, r, C_RT2, :], 31, 30),
                            op=ALU.add)
                        nc.vector.scalar_tensor_tensor(
                            out=ee[:, r], in0=dd[:, r], scalar=2.0,
                            in1=gp[:, r, C_X, 0:31],
                            op0=ALU.subtract, op1=ALU.mult)
                        nc.gpsimd.tensor_mul(s1[:, r], gp[:, r, C_RT, 0:31],
                                             cf[:, r, :, 0:31])
                        nc.gpsimd.tensor_mul(s2[:, r], gp[:, r, C_RT2, 0:31],
                                             cb[:, r, :, 0:31])

                    # scratch-only ops: (r, c) dims are contiguous -> merge
                    m = lambda ap: ap.rearrange("p r c t -> p (r c) t")
                    if "scan" not in skip:
                        nc.gpsimd.tensor_add(m(s1), m(s1),
                                             _rev_last(m(s2), 31, 30))
                        nc.vector.tensor_tensor(out=m(ot), in0=m(s1),
                                                in1=m(ee), op=ALU.subtract)

                    # single out DMA via Pool SWDGE: separate queue ring, so
                    # its wait on ot can't block next-pair HWDGE input loads
                    dst = bass.AP(
                        tensor=out_d.tensor,
                        offset=(b * HSLAB + h0) * W * COUT * T,
                        ap=[[COUT * T, W], [W * COUT * T, HC],
                            [1, COUT * T]])
                    if "dmaout" not in skip:
                        nc.gpsimd.dma_start(out=dst, in_=ot)
    nc.compile()
    return nc


_NC_CACHE = None


def _get_nc():
    global _NC_CACHE
    if _NC_CACHE is None:
        _NC_CACHE = build_nc()
    return _NC_CACHE


def make_host_inputs(x, conv_w, conv_b):
    """Pad x, permute/flatten weights; all bf16."""
    x = np.asarray(x, np.float32)
    conv_w = np.asarray(conv_w, np.float32)
    conv_b = np.asarray(conv_b, np.float32)
    bf = ml_dtypes.bfloat16

    xp = np.zeros((B, CIN, TP, H + 2, WP), np.float32)
    xp[:, :, 1:1 + T, 1:1 + H, 1:1 + W] = x
    xp = xp.astype(bf)

    wp = conv_w.reshape(6, COUT, CIN, 3, 3, 3)[GPERM].reshape(
        96, CIN, 3, 3, 3)
    bp = conv_b.reshape(6, COUT)[GPERM].reshape(96)

    wmain = np.zeros((128, 3, 96), np.float32)
    for g, (dh, dw) in enumerate(MAIN_TAPS):
        for dt in range(3):
            wmain[g * 16:(g + 1) * 16, dt, :] = wp[:, :, dt, dh + 1, dw + 1].T
    wleft = np.zeros((49, 96), np.float32)
    for dtg in range(3):
        wleft[dtg * 16:(dtg + 1) * 16, :] = wp[:, :, dtg, 2, 2].T
    wleft[48, :] = bp
    return xp, wmain.astype(bf), wleft.astype(bf)


def core_inputs(xp, wmain, wleft, k):
    """Pre-expand the exact SBUF images of the per-chunk mrep/lrep tiles."""
    bf = ml_dtypes.bfloat16
    # core slab rows: k*HSLAB .. k*HSLAB+HSLAB+1 of the padded (H+2) axis
    slab = xp[:, :, :, k * HSLAB:k * HSLAB + HSLAB + 2, :]  # [B,16,TP,18,130]

    mimg = np.zeros((NCH, 128, TP, RPITCH), bf)
    limg = np.zeros((NCH, 49, T, RPITCH), bf)
    for b in range(B):
        for ch in range(NCHUNK):
            pidx = b * NCHUNK + ch
            h0 = ch * HC
            for g, (dh, dw) in enumerate(MAIN_TAPS):
                # rows h0+1+dh, h0+2+dh flattened into the 260-run
                rows = slab[b, :, :, h0 + 1 + dh:h0 + 3 + dh, :]
                mimg[pidx, g * 16:(g + 1) * 16, :, 1 - dw:1 - dw + 2 * WP] = \
                    rows.reshape(CIN, TP, 2 * WP)
            for dtg in range(3):
                rows = slab[b, :, dtg:dtg + T, h0 + 2:h0 + 4, :]
                limg[pidx, dtg * 16:(dtg + 1) * 16, :, 0:2 * WP] = \
                    rows.reshape(CIN, T, 2 * WP)
            limg[pidx, 48, :, :] = bf(1.0)
    return {
        "inp": np.concatenate([mimg.ravel(), limg.ravel(), wmain.ravel(),
                               wleft.ravel()]),
    }


def kernel(x, conv_w, conv_b):
    nc = _get_nc()
    hi = make_host_inputs(x, conv_w, conv_b)
    in_maps = [core_inputs(*hi, k) for k in range(NCORES)]
    res = run_bass_kernel_spmd(nc, in_maps, list(range(NCORES)))
    outs = [res.results[k]["out"].transpose(0, 3, 4, 1, 2)
            for k in range(NCORES)]
    return np.concatenate(outs, axis=3).astype(np.float32)
